# revision 28
# baseline (speedup 1.0000x reference)
"""2-layer GCN (GCNConv x2, relu between) on 8 Trainium2 NeuronCores.

v4 design (x-space gather for L1, zero-startup, batched descriptors):
  - Nodes partitioned into 8 contiguous shards by dst; each core owns the
    edges incident (by dst) to its shard.
  - L1 gathers the INPUT rows x[src] (host-prescaled by dinv_src, bf16,
    exactly 256B/row) directly from two replicated input tables (split at
    node 32768 for int16 gather indices). No bulk x@W1 stage, no h1
    AllGather: gathers start at t~0. Aggregation runs in x-space
    (S_x[128, dst] = sum of gathered rows via 0/1 one-hot matmuls); the
    W1 transform is applied once per 128-dst window, with the self-loop
    term folded in as a second accumulating matmul (identity x h1p rows,
    h1p from a tiny own-shard stage0).
  - L1 is single-pass: each window's part-A and part-B tiles accumulate
    into one PSUM tile, tail follows immediately (x dinv_dst, +b1, relu,
    @W2, x dinv_dst) producing the 256B-padded h2 shard rows.
  - AllGather(h2 slab A = first 4096 rows/shard) fires after window 31's
    tail (~65% into L1); AllGather(slab B = 2154 rows) after L1. 8*4096 =
    32768 rows keeps part-A gather indices within int16.
  - L2 is the two-phase gather/segment-sum of the transformed h2 rows:
    phase A (parked in SBUF) overlaps the AG-h2B; phase B re-adds the
    parked partials and writes the output.
  - dma_gather calls are packed to the 1024-descriptor SWDGE ring cap
    (8 tiles/call, 994ns fixed cost each), round-robin over 4 queues.
  - One-hots are built batched (one DVE tensor_tensor per chunk-part,
    stride-0 broadcast APs) against a [128, SPAN] iota; tokens grouped
    into dst-intervals of width <= SPAN so each tile's one-hot spans SPAN
    columns. Pads (dstv=-1) gather RANDOM rows to spread HBM channels.
"""

import sys

if "/opt/trn_rl_repo" not in sys.path:
    sys.path.insert(0, "/opt/trn_rl_repo")

import numpy as np
import ml_dtypes

P = 128
SPAN = 64
KMAX = 4
CHUNK_W = 4
GMAX = 8  # tiles per dma_gather call (1024 descs; fine with the 3072-desc
          # SWDGE ring from dynamic_dma_scratch_size=49152)
SINGLE_PACKET = True
NSWQ = 4

N, E, IN, HID, OUT = 50000, 800000, 128, 64, 32
N_CORES = 8
SH = N // N_CORES  # 6250
W = (SH + P - 1) // P  # 49
XSPLIT = 32768  # L1 gather tables: x rows [0, XSPLIT) and [XSPLIT, N)
NXB = N - XSPLIT  # 17232
WA2 = 24  # h2 slab A windows (8*24*128 = 24576 rows, idx < 32768);
          # small enough that AG-h2A fires ~half-way through L1
RA2 = WA2 * P  # 4096
RB2 = SH - RA2  # 2154
BF16 = ml_dtypes.bfloat16


def _cdiv(a, b):
    return (a + b - 1) // b


def _build_intervals(counts):
    """counts: [N_CORES, W, 2, P] -> {(w, g): [(lo, hi, d0, ntiles)]}.

    DP-optimal interval boundaries: minimize total tiles (= padded
    gather descriptors) subject to each interval fitting a SPAN-wide
    one-hot window starting at d0e = min(lo & ~1, P - SPAN).
    """
    intervals = {}
    INF = 10**9
    for w in range(W):
        for gg in (0, 1):
            cnt = counts[:, w, gg, :]  # [8, 128]
            pref = np.zeros((N_CORES, P + 1), np.int64)
            pref[:, 1:] = np.cumsum(cnt, axis=1)
            C = [INF] * (P + 1)
            arg = [0] * (P + 1)
            C[0] = 0
            for end in range(1, P + 1):
                if (pref[:, end] - pref[:, end - 1]).sum() == 0 \
                        and C[end - 1] < INF:
                    # empty column: extend previous cover for free
                    C[end] = C[end - 1]
                    arg[end] = -1  # marker: column skipped
                for start in range(max(0, end - SPAN), end):
                    d0e = min(start & ~1, P - SPAN)
                    if end - d0e > SPAN or C[start] >= INF:
                        continue
                    mx = int((pref[:, end] - pref[:, start]).max())
                    t = _cdiv(mx, P) if mx > 0 else 0
                    if C[start] + t < C[end]:
                        C[end] = C[start] + t
                        arg[end] = start
            # traceback
            ivs = []
            end = P
            while end > 0:
                if arg[end] == -1:
                    end -= 1
                    continue
                start = arg[end]
                mx = int((pref[:, end] - pref[:, start]).max())
                if mx > 0:
                    d0e = min(start & ~1, P - SPAN)
                    ivs.append((start, end, d0e, _cdiv(mx, P)))
                end = start
            ivs.reverse()
            intervals[(w, gg)] = ivs
    return intervals


def _build_tiles_l1(intervals):
    """Single-pass chunks: per window group, part-A tiles then part-B."""
    tiles = []
    chunks = []
    win_tiles = {(w, gg): [] for w in range(W) for gg in (0, 1)}
    for c0 in range(0, W, CHUNK_W):
        ws = list(range(c0, min(c0 + CHUNK_W, W)))
        ch = {"t0": len(tiles), "ws": ws, "parts": []}
        for gg in (0, 1):
            p0 = len(tiles)
            for w in ws:
                for d0s, end, d0e, nt in intervals[(w, gg)]:
                    for k in range(nt):
                        tiles.append({"w": w, "g": gg, "d0": d0e,
                                      "lo": d0s, "hi": end, "slot": k})
                        win_tiles[(w, gg)].append(len(tiles) - 1)
            if len(tiles) > p0:
                ch["parts"].append((gg, p0, len(tiles)))
        ch["t1"] = len(tiles)
        if ch["t1"] > ch["t0"]:
            chunks.append(ch)
    return tiles, chunks, win_tiles


def _build_tiles_l2(intervals):
    """Phase-major chunks (all part-A chunks, then all part-B)."""
    tiles = []
    chunks = []
    win_tiles = {(w, gg): [] for w in range(W) for gg in (0, 1)}
    for gg in (0, 1):
        for c0 in range(0, W, CHUNK_W):
            ws = list(range(c0, min(c0 + CHUNK_W, W)))
            ch = {"t0": len(tiles), "ws": ws, "part": gg}
            for w in ws:
                for d0s, end, d0e, nt in intervals[(w, gg)]:
                    for k in range(nt):
                        tiles.append({"w": w, "g": gg, "d0": d0e,
                                      "lo": d0s, "hi": end, "slot": k})
                        win_tiles[(w, gg)].append(len(tiles) - 1)
            ch["t1"] = len(tiles)
            if ch["t1"] > ch["t0"]:
                chunks.append(ch)
    return tiles, chunks, win_tiles


def _pack_tokens(key_all, row_all, dl_all, core, tiles, win_tiles, pad_max,
                 seed):
    """Per-core token arrays (idx table + dst values) for one layer."""
    NT = len(tiles)
    per_core = []
    for c in range(N_CORES):
        sel = core == c
        key_c = key_all[sel]
        row_c = row_all[sel]
        dl_c = dl_all[sel]
        order = np.argsort(key_c, kind="stable")
        key_s = key_c[order]
        idx_s = row_c[order]
        dl_s = dl_c[order]

        rng = np.random.default_rng(seed + c)
        tok_idx = rng.integers(0, pad_max, size=NT * P).astype(np.int32)
        tok_dstv = np.full(NT * P, -1.0, np.float32)
        for w in range(W):
            for gg in (0, 1):
                base_key = (w * 2 + gg) * P
                tlist = win_tiles[(w, gg)]
                i = 0
                while i < len(tlist):
                    t0 = tlist[i]
                    lo_dl, hi_dl, d0e = (tiles[t0]["lo"], tiles[t0]["hi"],
                                         tiles[t0]["d0"])
                    ntk = 1
                    while (i + ntk < len(tlist)
                           and tiles[tlist[i + ntk]]["lo"] == lo_dl
                           and tiles[tlist[i + ntk]]["slot"] == ntk):
                        ntk += 1
                    a = np.searchsorted(key_s, base_key + lo_dl)
                    b = np.searchsorted(key_s, base_key + hi_dl)
                    n_c = b - a
                    for k in range(ntk):
                        tid = tlist[i + k]
                        s0 = a + k * P
                        m = max(0, min(P, n_c - k * P))
                        if m > 0:
                            o = tid * P
                            tok_idx[o : o + m] = idx_s[s0 : s0 + m]
                            tok_dstv[o : o + m] = (dl_s[s0 : s0 + m] - d0e)
                    i += ntk

        assert tok_idx.max() < 32768 and tok_idx.min() >= 0
        i16 = tok_idx.astype(np.int16).reshape(NT * P // 16, 16).T
        i16 = np.tile(i16, (8, 1)).copy()  # [128, NT*8]
        dstv_sb = tok_dstv.reshape(NT, P).T.astype(BF16).copy()  # [128, NT]
        per_core.append({"src16": i16, "dstv": dstv_sb})
    return per_core


def preprocess(edge_index):
    """Host-side graph preprocessing -> (meta, per_core arrays)."""
    src = edge_index[0].astype(np.int64)
    dst = edge_index[1].astype(np.int64)

    deg = (np.bincount(dst, minlength=N) + 1.0).astype(np.float32)
    dinv = (1.0 / np.sqrt(deg)).astype(np.float32)

    core = dst // SH
    dloc = dst % SH
    wv = dloc >> 7
    dl = dloc & 127

    # ---- layer 1: src part split at global node XSPLIT ----
    g1 = (src >= XSPLIT).astype(np.int64)
    row1 = np.where(g1 == 0, src, src - XSPLIT)
    counts1 = np.zeros((N_CORES, W, 2, P), np.int64)
    np.add.at(counts1, (core, wv, g1, dl), 1)
    iv1 = _build_intervals(counts1)
    tiles1, chunks1, wt1 = _build_tiles_l1(iv1)

    # ---- layer 2: src part split at shard-local row RA2 ----
    s_core = src // SH
    s_r = src % SH
    g2 = (s_r >= RA2).astype(np.int64)
    row2 = np.where(g2 == 0, s_core * RA2 + s_r, s_core * RB2 + (s_r - RA2))
    counts2 = np.zeros((N_CORES, W, 2, P), np.int64)
    np.add.at(counts2, (core, wv, g2, dl), 1)
    iv2 = _build_intervals(counts2)
    tiles2, chunks2, wt2 = _build_tiles_l2(iv2)

    key1 = ((wv * 2 + g1) * P + dl)
    key2 = ((wv * 2 + g2) * P + dl)
    pc1 = _pack_tokens(key1, row1, dl, core, tiles1, wt1,
                       min(XSPLIT, NXB), 1234)
    pc2 = _pack_tokens(key2, row2, dl, core, tiles2, wt2,
                       min(N_CORES * RA2, N_CORES * RB2), 5678)

    per_core = []
    for c in range(N_CORES):
        dpad = np.ones(W * P, np.float32)
        dpad[:SH] = dinv[c * SH : (c + 1) * SH]
        dinvbc = np.tile(dpad[None, :], (P, 1)).copy()  # [128, W*128] f32
        per_core.append({"src16_1": pc1[c]["src16"], "dstv_1": pc1[c]["dstv"],
                         "src16_2": pc2[c]["src16"], "dstv_2": pc2[c]["dstv"],
                         "dinvbc": dinvbc})

    meta = {"NT1": len(tiles1), "tiles1": tiles1, "chunks1": chunks1,
            "wt1": wt1, "NT2": len(tiles2), "tiles2": tiles2,
            "chunks2": chunks2, "wt2": wt2, "dinv": dinv,
            "tsp1": chunks1[0]["t1"]}
    return meta, per_core


IN_NAMES = ["xgA", "xgB", "xsT", "w1b", "w2b", "b1col", "b2bc", "iota",
            "i128b", "dinvbc", "src16_1a", "src16_1b", "dstv_1", "src16_2",
            "dstv_2"]


def make_inputs(x, W1, b1, W2, b2, meta, per_core):
    dinv = meta["dinv"]
    tsp1 = meta["tsp1"]
    xf = np.asarray(x, np.float32)
    xg = (xf * dinv[:, None]).astype(BF16)  # prescaled rows, 256B each
    xgA = np.ascontiguousarray(xg[:XSPLIT])
    xgB = np.ascontiguousarray(xg[XSPLIT:])

    iota = np.tile(np.arange(SPAN, dtype=np.float32)[None, :], (P, 1)).astype(BF16)
    i128b = np.eye(P, dtype=np.float32).astype(BF16)
    b1col = np.asarray(b1, np.float32).reshape(HID, 1)
    b2bc = np.tile(np.asarray(b2, np.float32)[None, :], (P, 1)).astype(BF16)
    w1b = np.asarray(W1, np.float32).astype(BF16)
    w2b = np.asarray(W2, np.float32).astype(BF16)
    ins_list = []
    for c, pc in enumerate(per_core):
        xsT = np.zeros((P, W * P), np.float32)
        xsT[:, :SH] = (xf[c * SH : (c + 1) * SH, :]
                       * dinv[c * SH : (c + 1) * SH, None]).T
        ins_list.append([
            xgA,
            xgB,
            xsT.astype(BF16),
            w1b,
            w2b,
            b1col,
            b2bc,
            iota,
            i128b,
            pc["dinvbc"],
            np.ascontiguousarray(pc["src16_1"][:, : tsp1 * 8]),
            np.ascontiguousarray(pc["src16_1"][:, tsp1 * 8 :]),
            pc["dstv_1"],
            pc["src16_2"],
            pc["dstv_2"],
        ])
    return ins_list


def build_kernel(tc, outs, ins, meta):
    from concourse import mybir

    nc = tc.nc
    (xgA_ap, xgB_ap, xsT_ap, w1_ap, w2_ap, b1_ap, b2_ap, iota_ap, i128b_ap,
     dinvbc_ap, src16_1a_ap, src16_1b_ap, dstv_1_ap, src16_2_ap,
     dstv_2_ap) = ins
    tsp1 = meta["tsp1"]
    out_ap = outs[0]

    NT1, NT2 = meta["NT1"], meta["NT2"]
    tiles1, chunks1, wt1 = meta["tiles1"], meta["chunks1"], meta["wt1"]
    tiles2, chunks2, wt2 = meta["tiles2"], meta["chunks2"], meta["wt2"]
    groups = [list(range(N_CORES))]

    f32 = mybir.dt.float32
    bf16 = mybir.dt.bfloat16
    i16 = mybir.dt.int16
    AT = mybir.ActivationFunctionType
    OP = mybir.AluOpType

    MAX_CT = max(
        max(ch["t1"] - ch["t0"] for ch in chunks1),
        max(ch["t1"] - ch["t0"] for ch in chunks2),
    )

    import contextlib

    with contextlib.ExitStack() as ctx:
        const = ctx.enter_context(tc.tile_pool(name="const", bufs=1))
        dram = ctx.enter_context(tc.tile_pool(name="dram", bufs=1, space="DRAM"))
        ohp = ctx.enter_context(tc.tile_pool(name="oh", bufs=2))
        tokp = ctx.enter_context(tc.tile_pool(name="tokp", bufs=2))
        tailp = ctx.enter_context(tc.tile_pool(name="tail", bufs=4))
        persist = ctx.enter_context(tc.tile_pool(name="persist", bufs=1))

        # gather index tables first: the first gather calls wait only on
        # the small chunk-0 slice, so gathers start within ~2us
        src16_1a_sb = const.tile([P, tsp1 * 8], i16)
        nc.sync.dma_start(src16_1a_sb[:], src16_1a_ap[:])
        dstv_1_sb = const.tile([P, NT1], bf16)
        nc.sync.dma_start(dstv_1_sb[:], dstv_1_ap[:])
        iota_sb = const.tile([P, SPAN], bf16)
        nc.sync.dma_start(iota_sb[:], iota_ap[:])
        w1_sb = const.tile([IN, HID], bf16)
        nc.sync.dma_start(w1_sb[:], w1_ap[:])
        w2_sb = const.tile([HID, OUT], bf16)
        nc.sync.dma_start(w2_sb[:], w2_ap[:])
        b1_sb = const.tile([HID, 1], f32)
        nc.sync.dma_start(b1_sb[:], b1_ap[:])
        b2_sb = const.tile([P, OUT], bf16)
        nc.sync.dma_start(b2_sb[:], b2_ap[:])
        i128b_sb = const.tile([P, P], bf16)
        nc.sync.dma_start(i128b_sb[:], i128b_ap[:])
        src16_1b_sb = const.tile([P, (NT1 - tsp1) * 8], i16)
        nc.sync.dma_start(src16_1b_sb[:], src16_1b_ap[:])
        dinvbc_sb = const.tile([P, W * P], f32)
        nc.sync.dma_start(dinvbc_sb[:], dinvbc_ap[:])
        src16_2_sb = const.tile([P, NT2 * 8], i16)
        nc.sync.dma_start(src16_2_sb[:], src16_2_ap[:])
        dstv_2_sb = const.tile([P, NT2], bf16)
        nc.sync.dma_start(dstv_2_sb[:], dstv_2_ap[:])
        xsT_sb = const.tile([P, W * P], bf16)
        nc.sync.dma_start(xsT_sb[:], xsT_ap[:])
        zrow_sb = const.tile([1, P], bf16)
        nc.vector.memset(zrow_sb[:], 0.0)

        # persistent per-window self-loop rows
        h1p_sb = persist.tile([P, W, HID], bf16)
        h2p_sb = persist.tile([P, W, OUT], bf16)

        # DRAM scratch (rows padded to 256B for dma_gather)
        h2_shardA = dram.tile([RA2, P], bf16)
        h2_shardB = dram.tile([RB2, P], bf16)
        h2xA = dram.tile([N_CORES * RA2, P], bf16)
        h2xB = dram.tile([N_CORES * RB2, P], bf16)
        warm_in = dram.tile([1, P], bf16)
        warm_out = dram.tile([N_CORES, P], bf16)

        qctr = [0]

        # warm up the collective stream (absorbs the one-time ~66us barrier
        # while L1 runs)
        nc.gpsimd.collective_compute(
            "AllGather", mybir.AluOpType.bypass, replica_groups=groups,
            ins=[warm_in[:]], outs=[warm_out[:]],
        )

        # PSUM is bank-granular (8 banks): 3 accumulator + 5 aux banks.
        psum_acc = ctx.enter_context(
            tc.tile_pool(name="psum_acc", bufs=3, space="PSUM"))
        psum_aux = ctx.enter_context(
            tc.tile_pool(name="psum_aux", bufs=5, space="PSUM"))

        # ------- stage0-lite: own-shard h1p rows for self-loop inits -------
        # h1p[i] = (dinv_i * x_i) @ W1 from the prescaled xsT input.
        def stage0_win(w):
            dw = min(P, SH - w * P)
            h1_ps = psum_aux.tile([P, HID], f32, tag="aux")
            nc.tensor.matmul(
                out=h1_ps[:dw, :],
                lhsT=xsT_sb[:, w * P : w * P + dw],
                rhs=w1_sb[:],
                start=True, stop=True,
            )
            nc.scalar.activation(
                out=h1p_sb[:dw, w, :], in_=h1_ps[:dw, :], func=AT.Copy,
            )

        def gather_tiles(t0, t1, src_hx, src16_sb):
            """Issue dma_gather calls for tiles [t0, t1), <=GMAX tiles each."""
            tokt = tokp.tile([P, MAX_CT, P], bf16, tag="tok")
            for ca in range(t0, t1, GMAX):
                cb = min(ca + GMAX, t1)
                nc.gpsimd.dma_gather(
                    out_ap=tokt[:, ca - t0 : cb - t0, :],
                    in_ap=src_hx[:, :],
                    idxs_ap=src16_sb[:, ca * 8 : cb * 8],
                    num_idxs=(cb - ca) * P,
                    num_idxs_reg=(cb - ca) * P,
                    elem_size=P,
                    single_packet=SINGLE_PACKET,
                    queue_num=qctr[0] % NSWQ,
                )
                qctr[0] += 1
            return tokt

        def onehot(ct, t0, dstv_sb):
            oh = ohp.tile([P, MAX_CT, SPAN], bf16, tag="oh")
            nc.vector.tensor_tensor(
                out=oh[:, :ct, :],
                in0=dstv_sb[:, t0 : t0 + ct].unsqueeze(2)
                    .broadcast_to([P, ct, SPAN]),
                in1=iota_sb[:].unsqueeze(1).broadcast_to([P, ct, SPAN]),
                op=OP.is_equal,
            )
            return oh

        # ---------------- L1: single pass over windows ----------------
        ag2a_done = [False]

        def l1_tail(w, dw, SB):
            """(dinv_dst *) relu(+b1) @ W2 * dinv_dst -> h2p + h2 shard."""
            t1 = tailp.tile([HID, P], f32, tag="t1")
            nc.vector.tensor_tensor(
                out=t1[:], in0=SB[:, :],
                in1=dinvbc_sb[:HID, w * P : (w + 1) * P], op=OP.mult,
            )
            ut = tailp.tile([HID, P], bf16, tag="ut")
            nc.scalar.activation(
                out=ut[:], in_=t1[:], func=AT.Relu, bias=b1_sb[:, 0:1],
            )
            h2T_ps = psum_aux.tile([OUT, P], f32, tag="aux")
            nc.tensor.matmul(
                out=h2T_ps[:], lhsT=w2_sb[:], rhs=ut[:],
                start=True, stop=True,
            )
            h2T_sb = tailp.tile([OUT, P], bf16, tag="h2Ts")
            nc.vector.tensor_tensor(
                out=h2T_sb[:], in0=h2T_ps[:],
                in1=dinvbc_sb[:OUT, w * P : (w + 1) * P], op=OP.mult,
            )
            h2p_ps = psum_aux.tile([P, OUT], bf16, tag="aux")
            nc.tensor.transpose(h2p_ps[:], h2T_sb[:], i128b_sb[:OUT, :OUT])
            nc.scalar.activation(
                out=h2p_sb[:dw, w, :], in_=h2p_ps[:dw, :], func=AT.Copy,
            )
            if w < WA2:
                nc.sync.dma_start(h2_shardA[w * P : w * P + dw, 0:OUT],
                                  h2p_sb[:dw, w, :])
            else:
                r0 = w * P - RA2
                nc.sync.dma_start(h2_shardB[r0 : r0 + dw, 0:OUT],
                                  h2p_sb[:dw, w, :])

        # stage0-lite first: no gather deps; overlaps the first chunks'
        # gathers on DMA/GpSimd while Tensor/Scalar burn ~10us.
        for w in range(W):
            stage0_win(w)

        for ci, ch in enumerate(chunks1):
            t0 = ch["t0"]
            ct = ch["t1"] - t0
            # one token tile per chunk; separate gather calls per part table
            tokt = tokp.tile([P, MAX_CT, P], bf16, tag="tok")
            for gg, p0, p1 in ch["parts"]:
                src_hx = xgA_ap if gg == 0 else xgB_ap
                for ca in range(p0, p1, GMAX):
                    cb = min(ca + GMAX, p1)
                    if cb <= tsp1:
                        idxs = src16_1a_sb[:, ca * 8 : cb * 8]
                    else:
                        idxs = src16_1b_sb[:, (ca - tsp1) * 8
                                           : (cb - tsp1) * 8]
                    nc.gpsimd.dma_gather(
                        out_ap=tokt[:, ca - t0 : cb - t0, :],
                        in_ap=src_hx[:, :],
                        idxs_ap=idxs,
                        num_idxs=(cb - ca) * P,
                        num_idxs_reg=(cb - ca) * P,
                        elem_size=P,
                        single_packet=SINGLE_PACKET,
                        queue_num=qctr[0] % NSWQ,
                    )
                    qctr[0] += 1
            oh = onehot(ct, t0, dstv_1_sb)
            for w in ch["ws"]:
                dw = min(P, SH - w * P)
                wtiles = wt1[(w, 0)] + wt1[(w, 1)]
                # x-space accumulation S_x[128, dst]
                Sx = psum_acc.tile([P, P], f32, tag="acc", name=f"Sx_{w}")
                if not wtiles:
                    nc.tensor.matmul(
                        out=Sx[:, :], lhsT=zrow_sb[:, 0:P],
                        rhs=zrow_sb[:, 0:P], start=True, stop=True,
                        skip_group_check=True,
                    )
                for j, tid in enumerate(wtiles):
                    tm = tiles1[tid]
                    d0 = tm["d0"]
                    nc.tensor.matmul(
                        out=Sx[:, d0 : d0 + SPAN],
                        lhsT=tokt[:, tid - t0, :],
                        rhs=oh[:, tid - t0, :],
                        start=(j == 0), stop=(j == len(wtiles) - 1),
                        skip_group_check=True,
                    )
                sxb = tailp.tile([P, P], bf16, tag="sxb")
                nc.scalar.activation(
                    out=sxb[:, :dw], in_=Sx[:, :dw], func=AT.Copy,
                )
                # transform + self-loop: h1w = h1p_win^T (identity) + W1^T Sx
                h1w_ps = psum_aux.tile([HID, P], f32, tag="aux",
                                       name=f"h1w_{w}")
                nc.tensor.matmul(
                    out=h1w_ps[:, :dw],
                    lhsT=h1p_sb[:dw, w, :],
                    rhs=i128b_sb[:dw, :dw],
                    start=True, stop=False,
                    skip_group_check=True,
                )
                nc.tensor.matmul(
                    out=h1w_ps[:, :dw],
                    lhsT=w1_sb[:],
                    rhs=sxb[:, :dw],
                    start=False, stop=True,
                    skip_group_check=True,
                )
                l1_tail(w, dw, h1w_ps)
            if ch["ws"][-1] >= WA2 - 1 and not ag2a_done[0]:
                ag2a_done[0] = True
                nc.gpsimd.collective_compute(
                    "AllGather", mybir.AluOpType.bypass,
                    replica_groups=groups,
                    ins=[h2_shardA[:]], outs=[h2xA[:]],
                )

        nc.gpsimd.collective_compute(
            "AllGather", mybir.AluOpType.bypass, replica_groups=groups,
            ins=[h2_shardB[:]], outs=[h2xB[:]],
        )

        # ---------------- L2: two phases over src parts ----------------
        with tc.tile_pool(name="sa2", bufs=1) as sap:
            SA_sb = sap.tile([OUT, W * P], f32)
            for ch in chunks2:
                part = ch["part"]
                t0 = ch["t0"]
                ct = ch["t1"] - t0
                src_hx = h2xA if part == 0 else h2xB
                tokt = gather_tiles(t0, ch["t1"], src_hx, src16_2_sb)
                oh = onehot(ct, t0, dstv_2_sb)
                for w in ch["ws"]:
                    dw = min(P, SH - w * P)
                    wtiles = wt2[(w, part)]
                    S = psum_acc.tile([OUT, P], f32, tag="acc",
                                      name=f"S2_{part}_{w}")
                    if part == 0:
                        nc.tensor.matmul(
                            out=S[:, :], lhsT=zrow_sb[:, 0:OUT],
                            rhs=zrow_sb[:, 0:P],
                            start=True, stop=(len(wtiles) == 0),
                            skip_group_check=True,
                        )
                    else:
                        nc.tensor.matmul(
                            out=S[:, :],
                            lhsT=h2p_sb[:dw, w, :],
                            rhs=i128b_sb[:dw, :],
                            start=True, stop=(len(wtiles) == 0),
                            skip_group_check=True,
                        )
                    for j, tid in enumerate(wtiles):
                        tm = tiles2[tid]
                        d0 = tm["d0"]
                        nc.tensor.matmul(
                            out=S[:, d0 : d0 + SPAN],
                            lhsT=tokt[:, tid - t0, 0:OUT],
                            rhs=oh[:, tid - t0, :],
                            start=False, stop=(j == len(wtiles) - 1),
                            skip_group_check=True,
                        )
                    if part == 0:
                        nc.scalar.activation(
                            out=SA_sb[:, w * P : (w + 1) * P], in_=S[:, :],
                            func=AT.Copy,
                        )
                    else:
                        t0v = tailp.tile([OUT, P], f32, tag="t0")
                        nc.vector.tensor_tensor(
                            out=t0v[:], in0=S[:, :],
                            in1=SA_sb[:, w * P : (w + 1) * P], op=OP.add,
                        )
                        o1 = tailp.tile([OUT, P], bf16, tag="o1")
                        nc.vector.tensor_tensor(
                            out=o1[:], in0=t0v[:],
                            in1=dinvbc_sb[:OUT, w * P : (w + 1) * P],
                            op=OP.mult,
                        )
                        o2_ps = psum_aux.tile([P, OUT], bf16, tag="aux")
                        nc.tensor.transpose(o2_ps[:], o1[:],
                                            i128b_sb[:OUT, :OUT])
                        o3 = tailp.tile([P, OUT], f32, tag="o3")
                        nc.vector.tensor_tensor(
                            out=o3[:dw, :], in0=o2_ps[:dw, :],
                            in1=b2_sb[:dw, :], op=OP.add,
                        )
                        nc.sync.dma_start(out_ap[w * P : w * P + dw, :],
                                          o3[:dw, :])


def compile_kernel(x, W1, b1, W2, b2, edge_index):
    """Build + compile. Returns (nc, in_maps, meta)."""
    import concourse.tile as tile
    from concourse import bacc, mybir

    meta, per_core = preprocess(np.asarray(edge_index))
    ins_list = make_inputs(x, W1, b1, W2, b2, meta, per_core)

    nc = bacc.Bacc(
        "TRN2", target_bir_lowering=False, debug=False, num_devices=N_CORES,
        num_swdge_queues=NSWQ, dynamic_dma_scratch_size=49152,
    )
    in_aps = []
    for nm, a in zip(IN_NAMES, ins_list[0]):
        in_aps.append(
            nc.dram_tensor(nm, list(a.shape), mybir.dt.from_np(a.dtype),
                           kind="ExternalInput").ap()
        )
    out_t = nc.dram_tensor("out", [SH, OUT], mybir.dt.float32,
                           kind="ExternalOutput")
    with tile.TileContext(nc) as tc:
        build_kernel(tc, [out_t.ap()], in_aps, meta)
    nc.compile()

    in_maps = [
        {nm: np.ascontiguousarray(a) for nm, a in zip(IN_NAMES, arrs)}
        for arrs in ins_list
    ]
    return nc, in_maps, meta


def run(x, W1, b1, W2, b2, edge_index, trace=False, ntff=False, tmpdir=None):
    from concourse import bass_utils
    from concourse.bass_interp import get_hw_module

    nc, in_maps, meta = compile_kernel(x, W1, b1, W2, b2, edge_index)
    old_m = nc.m
    nc.m = get_hw_module(nc.m)
    try:
        res = bass_utils.run_bass_kernel_spmd(
            nc, in_maps, core_ids=list(range(N_CORES)), trace=ntff,
            tmpdir=tmpdir,
        )
        bench_ns = _bench(nc, in_maps, N_CORES) if trace else None
    finally:
        nc.m = old_m
    out = np.concatenate([res.results[c]["out"] for c in range(N_CORES)], axis=0)
    return out, res, bench_ns


def _bench(nc, in_maps, n_cores, iters=30):
    """Interleaved wall-clock benchmark (upper bound on HW time)."""
    import time

    import jax
    from concourse import bass2jax
    from jax.sharding import Mesh, PartitionSpec
    from jax.experimental.shard_map import shard_map

    part_name = nc.partition_id_tensor.name if nc.partition_id_tensor else None
    in_names, out_names, out_avals, zero_outs = [], [], [], []
    for alloc in nc.m.functions[0].allocations:
        if not isinstance(alloc, bass2jax.mybir.MemoryLocationSet):
            continue
        name = alloc.memorylocations[0].name
        if alloc.kind == "ExternalInput":
            if name != part_name:
                in_names.append(name)
        elif alloc.kind == "ExternalOutput":
            out_names.append(name)
            shape = tuple(alloc.tensor_shape)
            dtype = bass2jax.mybir.dt.np(alloc.dtype)
            out_avals.append(jax.core.ShapedArray(shape, dtype))
            zero_outs.append(np.zeros(shape, dtype))
    n_params = len(in_names)
    all_names = in_names + out_names
    if part_name is not None:
        all_names = all_names + [part_name]

    def _body(*args):
        ins = list(args[:n_params])
        outs = list(args[n_params:])
        operands = ins + outs
        if part_name is not None:
            operands.append(bass2jax.partition_id_tensor())
        outs = list(
            bass2jax._bass_exec_p.bind(
                *operands,
                out_avals=tuple(out_avals),
                in_names=tuple(all_names),
                out_names=tuple(out_names),
                lowering_input_output_aliases=(),
                sim_require_finite=True,
                sim_require_nnan=True,
                nc=nc,
            )
        )
        return tuple(outs)

    devices = jax.devices()[:n_cores]
    mesh = Mesh(np.asarray(devices), ("core",))
    nio = n_params + len(out_names)
    sh = jax.sharding.NamedSharding(mesh, PartitionSpec("core"))
    concat_in = [
        jax.device_put(
            np.concatenate([in_maps[c][nm] for c in range(n_cores)], axis=0), sh
        )
        for nm in in_names
    ]
    concat_zero = [
        jax.device_put(np.zeros((n_cores * z.shape[0], *z.shape[1:]), z.dtype), sh)
        for z in zero_outs
    ]

    fn = jax.jit(
        shard_map(
            _body,
            mesh=mesh,
            in_specs=(PartitionSpec("core"),) * nio,
            out_specs=(PartitionSpec("core"),) * len(out_names),
            check_rep=False,
        ),
        keep_unused=True,
    )
    base_fn = jax.jit(lambda a: a[0:1, 0:1] * 2.0)
    jax.block_until_ready(fn(*concat_in, *concat_zero))
    jax.block_until_ready(base_fn(concat_in[0]))
    deltas = []
    for _ in range(iters):
        t0 = time.perf_counter()
        jax.block_until_ready(base_fn(concat_in[0]))
        t1 = time.perf_counter()
        jax.block_until_ready(fn(*concat_in, *concat_zero))
        t2 = time.perf_counter()
        jax.block_until_ready(base_fn(concat_in[0]))
        t3 = time.perf_counter()
        deltas.append((t2 - t1) - ((t1 - t0) + (t3 - t2)) / 2.0)
    deltas.sort()
    med = deltas[len(deltas) // 2]
    print(f"[bench] interleaved delta min={deltas[0]*1e6:.1f}us "
          f"median={med*1e6:.1f}us max={deltas[-1]*1e6:.1f}us")
    return int(max(0.0, med) * 1e9)


def kernel(x, W1, b1, W2, b2, edge_index):
    out, _, _ = run(
        np.asarray(x, np.float32),
        np.asarray(W1, np.float32),
        np.asarray(b1, np.float32),
        np.asarray(W2, np.float32),
        np.asarray(b2, np.float32),
        np.asarray(edge_index, np.int32),
    )
    return out
